# revision 26
# baseline (speedup 1.0000x reference)
"""Self-contained Trainium2 Bass kernel for nn_Attention (additive attention scores).

kernel(**inputs) takes FULL unsharded inputs and returns the FULL output:
  decoder_hide [32, 512] f32, encoder_out [32, 2048, 1024] f32, mask [32, 2048] i32,
  W_attn [1536, 512] f32, b_attn [512] f32, v_w [512] f32  ->  out [32, 2048] f32

Strategy: data-parallel over batch across 8 NeuronCores (4 batches/core),
weights replicated. v3 pipeline. Measured facts driving it: the PE sustains
only ~1.2GHz on this part (229ns for a 512-col fp8 DoubleRow matmul even
fully dense, vs 107ns at the nominal 2.4GHz), so the kernel is PE-cycle-bound
unless every PE stage is minimal; the pure-load floor is ~98us/pass.
  - encoder tiles loaded natural [s, e] with f32->bf16 cast during SWDGE DMA
  - bf16->fp8 convert (round-to-nearest) SPLIT between DVE and ACT by batch
    (conv_act) so neither engine owns the 16M-element conversion
  - fp8 PAIRS viewed as u16/bf16 and transposed on the PE (16 transposes of
    [128,128] per tile -- half the cycles of bf16 transposes), copied back
    PSUM->SBUF on DVE (GPSIMD cannot access PSUM)
  - proj: fp8 DoubleRow matmuls, K=256/call at 0.5 cyc/col; W_e pre-scaled
    x16 (e4m3 range), compensated via the ACT tanh scale=1/16
  - tanh outputs fp8 (RTN) into plane-pair tiles en8[p, i, s] (h = 256jv +
    128i + p), so the v-dot also runs as fp8 DoubleRow: 8 matmuls of 256
    cycles per s-block instead of 16 bf16 matmuls of 512 -- the PE drops from
    ~131k to ~106k cycles/pass, just under the DMA roof. v_w is kept in fp8
    x16 (VSCALE) and compensated for free via the exp scale=1/16 (the masked
    fill becomes -100000*16).
  - the per-s-block mask+exp drain is DEFERRED into the next s-block's b-loop
    (emitted at b==1) so the in-order ACT/DVE queues never stall behind the
    v-dot accumulation: no per-block pipeline bubble.
"""
from contextlib import ExitStack

import numpy as np

B, S, H, E = 32, 2048, 512, 1024
NCORES = 8
B_LOC = B // NCORES
SBLK = 512
WSCALE = 16.0
VSCALE = 16.0

_CACHE = {}


def _build_kernel(
    repeats=1,
    psbufs=2,
    dppbufs=2,
    trbufs=8,
    natbufs=6,
    enbufs=3,
    stage=4,
    conv_map=("v", "a", "v", "a"),
    fp8dot=True,
    burst_b=0,
    drain_b=1,
    pair=False,
):
    """stage: 1=loads+convert, 2=+transposes, 3=+proj matmuls, 4=full.
    conv_map[b]: engine for the bf16->fp8 convert of batch b -- "v" DVE,
    "a" ACT, "p" Pool/GPSIMD (all three cast with round-to-nearest).
    burst_b/drain_b: which b-iteration of the NEXT s-block emits the
    deferred v-dot burst / mask+exp drain."""
    import concourse.tile as tile
    from concourse import bacc, mybir

    F32 = mybir.dt.float32
    BF16 = mybir.dt.bfloat16
    FP8 = mybir.dt.float8e4
    U16 = mybir.dt.uint16
    I32 = mybir.dt.int32
    AF = mybir.ActivationFunctionType
    DR = mybir.MatmulPerfMode.DoubleRow

    NSB = S // SBLK
    NSUB = SBLK // 128
    EPB = E // 256  # e pair-blocks (256 e's contracted per DoubleRow matmul)
    NHC = H // 128
    NDC = H // 128
    NJV = NHC // 2  # h pair-blocks for the fp8 v-dot

    nc = bacc.Bacc("TRN2", target_bir_lowering=False, debug=False, num_devices=NCORES)

    dec = nc.dram_tensor("decoder_hide", [B_LOC, H], F32, kind="ExternalInput")
    enc = nc.dram_tensor("encoder_out", [B_LOC, S, E], F32, kind="ExternalInput")
    msk = nc.dram_tensor("mask", [B_LOC, S], I32, kind="ExternalInput")
    w_attn = nc.dram_tensor("W_attn", [3 * H, H], F32, kind="ExternalInput")
    b_attn = nc.dram_tensor("b_attn", [H], F32, kind="ExternalInput")
    v_w = nc.dram_tensor("v_w", [H], F32, kind="ExternalInput")
    out = nc.dram_tensor("out", [B_LOC, S], F32, kind="ExternalOutput")

    with ExitStack() as ctx:
        tc = ctx.enter_context(tile.TileContext(nc))
        singles = ctx.enter_context(tc.tile_pool(name="singles", bufs=1))
        natp = ctx.enter_context(tc.tile_pool(name="natp", bufs=natbufs))
        trp = ctx.enter_context(tc.tile_pool(name="trp", bufs=trbufs))
        enp = ctx.enter_context(tc.tile_pool(name="enp", bufs=enbufs))
        n8p = ctx.enter_context(tc.tile_pool(name="n8p", bufs=natbufs))
        psp = ctx.enter_context(tc.tile_pool(name="psp", bufs=psbufs, space="PSUM"))
        attp = ctx.enter_context(tc.tile_pool(name="attp", bufs=2, space="PSUM"))
        dpp = ctx.enter_context(tc.tile_pool(name="dpp", bufs=dppbufs, space="PSUM"))

        # ---- constants ----
        # W_e in DoubleRow interleave: wdr[p, pb, i, h] = WSCALE * W_e[256*pb + 2*p + i, h]
        wf = singles.tile([128, EPB, 2, H], F32)
        nc.gpsimd.dma_start(
            out=wf[:], in_=w_attn[H:, :].rearrange("(pb p i) h -> p pb i h", p=128, i=2)
        )
        wdr = singles.tile([128, EPB, 2, H], FP8)
        nc.vector.tensor_scalar_mul(wdr[:], wf[:], WSCALE)
        wh_f = singles.tile([128, NDC, H], F32)
        nc.sync.dma_start(
            out=wh_f[:], in_=w_attn[:H, :].rearrange("(dc p) h -> p dc h", p=128)
        )
        batt = singles.tile([128, NHC], F32)
        nc.sync.dma_start(out=batt[:], in_=b_attn.rearrange("(hc p) -> p hc", p=128))
        vwf = singles.tile([128, NHC], F32)
        nc.gpsimd.dma_start(out=vwf[:], in_=v_w.rearrange("(hc p) -> p hc", p=128))
        dect = singles.tile([128, NDC, B_LOC], F32)
        for dc in range(NDC):
            nc.gpsimd.dma_start(
                out=dect[:, dc, :],
                in_=dec[:, dc * 128 : (dc + 1) * 128].rearrange("b p -> p b"),
            )
        maskt = singles.tile([B_LOC, S], I32)
        nc.sync.dma_start(out=maskt[:], in_=msk[:, :])

        identu = singles.tile([128, 128], BF16)
        from concourse import masks

        masks.make_identity(nc, identu[:])

        if fp8dot:
            # v_w fp8 diag-pair tiles for the DoubleRow v-dot:
            # vz8k[p, b, jv, i, m] = VSCALE * v_w[256*jv + 128*i + p] at m==b.
            # One DISTINCT slice per group member (b, jv): a dual-fp8 DR
            # accumulation group whose members reuse the same lhsT AP 4+
            # times returns corrupted sums (measured); one-use-per-member is
            # clean. Column dim padded to 128: dual-fp8 Ldweights rejects
            # small stationary free dims (s3_lw_dual_fp8_restrictions); the
            # matmul cost is out-free-dim based, so the pad is free. Only
            # PSUM rows 0..B_LOC-1 are consumed downstream.
            vz8k = singles.tile([128, B_LOC, NJV, 2, 128], FP8)
            nc.vector.memset(vz8k[:], 0.0)
            for b in range(B_LOC):
                for jv in range(NJV):
                    for i in range(2):
                        hc = 2 * jv + i
                        nc.vector.tensor_scalar_mul(
                            vz8k[:, b, jv, i, b : b + 1], vwf[:, hc : hc + 1], VSCALE
                        )
        else:
            vw_bf = singles.tile([128, NHC], BF16)
            nc.vector.tensor_copy(vw_bf[:], vwf[:])
            vz = singles.tile([128, B_LOC, NHC, B_LOC], BF16)
            nc.vector.memset(vz[:], 0.0)
            for b in range(B_LOC):
                for hc in range(NHC):
                    nc.vector.tensor_copy(vz[:, b, hc, b : b + 1], vw_bf[:, hc : hc + 1])

        # ---- dec_proj bias: decb[:, hc, b] = W_h.T @ dec.T + b_attn ----
        decb = singles.tile([128, NHC, B_LOC], F32)
        for hc in range(NHC):
            dp = dpp.tile([128, B_LOC], F32)
            for dc in range(NDC):
                nc.tensor.matmul(
                    dp[:],
                    wh_f[:, dc, hc * 128 : (hc + 1) * 128],
                    dect[:, dc, :],
                    start=(dc == 0),
                    stop=(dc == NDC - 1),
                )
            nc.scalar.activation(
                decb[:, hc, :], dp[:], AF.Identity, bias=batt[:, hc : hc + 1]
            )

        L = singles.tile([B_LOC, S], F32)
        Ex = singles.tile([B_LOC, S], F32)
        Ssum4 = singles.tile([B_LOC, NSB], F32)

        mscale = VSCALE if fp8dot else 1.0

        def drain(prev):
            """Deferred per-s-block mask+exp; emitted during the NEXT s-block
            so the in-order ACT/DVE queues never stall on the v-dot accum."""
            attps_p, sb_p = prev
            Lms = singles.tile([B_LOC, SBLK], F32, name=f"Lms{sb_p % 2}")
            nc.vector.memset(Lms[:], -100000.0 * mscale)
            nc.vector.copy_predicated(
                Lms[:],
                maskt[:, sb_p * SBLK : (sb_p + 1) * SBLK],
                attps_p[0:B_LOC, :],
            )
            # logits bounded by sum|v_w| ~ 23: exp needs no max subtraction;
            # masked lanes get exp(-1e5) = 0. accum_out = partial row sums.
            nc.scalar.activation(
                Ex[:, sb_p * SBLK : (sb_p + 1) * SBLK],
                Lms[:],
                AF.Exp,
                accum_out=Ssum4[:, sb_p : sb_p + 1],
                scale=1.0 / mscale,
            )

        # ---- main loop over s-blocks ----
        import contextlib

        # For_i places an InstAllEngineBarrier in its per-iteration semaphore
        # reset block -- a full 5-engine drain+refill per trip. The real
        # single-pass kernel has no such barrier, so the timing builds unroll
        # 4 passes per trip to keep the loop overhead out of the slope.
        unroll = 4 if repeats > 1 and repeats % 4 == 0 else 1
        loop_ctx = (
            tc.For_i(0, repeats // unroll, 1) if repeats > 1 else contextlib.nullcontext()
        )
        def burst(prev_v):
            """The deferred v-dot matmuls for s-block sb_p, emitted as ONE
            uninterrupted 8-member dual-fp8 DR accumulation group: an open
            dual-fp8 group is corrupted if other matmuls run between its
            members (measured), so the whole group runs back-to-back, placed
            after the next s-block's first transposes so the last tanh has
            already landed and the PE never stalls."""
            en8_p, attps_p, _sb_p = prev_v
            for pb_ in range(B_LOC):
                for pjv in range(NJV):
                    nc.tensor.matmul(
                        attps_p[:],
                        vz8k[:, pb_, pjv],
                        en8_p[:, pb_, pjv],
                        start=(pb_ == 0 and pjv == 0),
                        stop=(pb_ == B_LOC - 1 and pjv == NJV - 1),
                        perf_mode=DR,
                    )

        with loop_ctx:
         prev = None
         prev_v = None
         for _u in range(unroll):
          for sb in range(NSB):
            if stage >= 4:
                attps = attp.tile([128 if fp8dot else B_LOC, SBLK], F32)
                if fp8dot:
                    # one fp8 energy tile per s-block: en8all[p, b, jv, i, s]
                    en8all = enp.tile([128, B_LOC, NJV, 2, SBLK], FP8)
            else:
                attps = None
            pending = []  # bf16 path: lagged v-dot matmuls (b, hc, en)

            def front(b):
                """load + convert + pair-transposes for batch b; returns tpr.
                Also hosts the deferred drain/burst of the previous s-block."""
                nonlocal prev, prev_v
                if b == drain_b and prev is not None:
                    drain(prev)
                    prev = None
                natf = natp.tile([128, NSUB, E], BF16, name="natf")
                nc.gpsimd.dma_start(
                    out=natf[:],
                    in_=enc[b, sb * SBLK : (sb + 1) * SBLK, :].rearrange(
                        "(sub p) e -> p sub e", p=128
                    ),
                )
                nat = n8p.tile([128, NSUB, E], FP8)
                ceng = conv_map[b]
                if ceng == "a":
                    nc.scalar.activation(nat[:], natf[:], AF.Identity)
                elif ceng == "p":
                    nc.gpsimd.tensor_copy(nat[:], natf[:])
                else:
                    nc.vector.tensor_copy(nat[:], natf[:])
                tpr = trp.tile([128, EPB, NSUB, 128], U16)
                nat16 = nat[:].bitcast(U16)
                for pb in range(EPB):
                    trps = dpp.tile([128, NSUB * 128], BF16)
                    for sub in range(NSUB):
                        nc.tensor.transpose(
                            trps[:, sub * 128 : (sub + 1) * 128],
                            nat16[:, sub, pb * 128 : (pb + 1) * 128].bitcast(BF16),
                            identu[:],
                        )
                    nc.vector.tensor_copy(tpr[:, pb, :, :], trps[:].bitcast(U16))
                if b == burst_b and fp8dot and prev_v is not None:
                    burst(prev_v)
                    prev = (prev_v[1], prev_v[2])
                    prev_v = None
                return tpr

            def projtanh(b, hc, tpr):
                """one proj accumulation group + tanh for (b, hc)."""
                ps = psp.tile([128, SBLK], F32)
                for ecp in range(EPB):
                    rhs = (
                        tpr[:, ecp, :, :]
                        .bitcast(FP8)
                        .rearrange("p sub (s i) -> p i (sub s)", i=2)
                    )
                    nc.tensor.matmul(
                        ps[:],
                        wdr[:, ecp, :, hc * 128 : (hc + 1) * 128],
                        rhs,
                        start=(ecp == 0),
                        stop=(ecp == EPB - 1),
                        perf_mode=DR,
                    )
                jv, ih = divmod(hc, 2)
                nc.scalar.activation(
                    en8all[:, b, jv, ih],
                    ps[:],
                    AF.Tanh,
                    bias=decb[:, hc, b : b + 1],
                    scale=1.0 / WSCALE,
                )

            if pair and stage >= 4 and fp8dot:
                for bp in range(0, B_LOC, 2):
                    tprs = {b: front(b) for b in (bp, bp + 1)}
                    for hc in range(NHC):
                        for b in (bp, bp + 1):
                            projtanh(b, hc, tprs[b])
                prev_v = (en8all, attps, sb)
                continue

            for b in range(B_LOC):
                if b == drain_b and prev is not None:
                    drain(prev)
                    prev = None
                natf = natp.tile([128, NSUB, E], BF16, name="natf")
                nc.gpsimd.dma_start(
                    out=natf[:],
                    in_=enc[b, sb * SBLK : (sb + 1) * SBLK, :].rearrange(
                        "(sub p) e -> p sub e", p=128
                    ),
                )
                # round-to-nearest bf16->fp8, split across engines by batch
                nat = n8p.tile([128, NSUB, E], FP8)
                ceng = conv_map[b]
                if ceng == "a":
                    nc.scalar.activation(nat[:], natf[:], AF.Identity)
                elif ceng == "p":
                    nc.gpsimd.tensor_copy(nat[:], natf[:])
                else:
                    nc.vector.tensor_copy(nat[:], natf[:])
                if stage < 2:
                    nc.vector.tensor_copy(
                        L[0:1, sb * SBLK + b : sb * SBLK + b + 1], nat[0:1, 0, 0:1]
                    )
                    continue
                # tpr[p, pb, sub, s0] = u16 pair (e=2*(pb*128+p), e+1) at s = sub*128+s0
                tpr = trp.tile([128, EPB, NSUB, 128], U16)
                nat16 = nat[:].bitcast(U16)
                # the u16 pair data rides through the PE transpose as bf16
                # (bit-identical view; Ldweights rejects integer dtypes)
                for pb in range(EPB):
                    trps = dpp.tile([128, NSUB * 128], BF16)
                    for sub in range(NSUB):
                        nc.tensor.transpose(
                            trps[:, sub * 128 : (sub + 1) * 128],
                            nat16[:, sub, pb * 128 : (pb + 1) * 128].bitcast(BF16),
                            identu[:],
                        )
                    nc.vector.tensor_copy(tpr[:, pb, :, :], trps[:].bitcast(U16))
                if b == burst_b and fp8dot and prev_v is not None:
                    burst(prev_v)
                    prev = (prev_v[1], prev_v[2])
                    prev_v = None
                if stage < 3:
                    nc.vector.tensor_copy(
                        L[0:1, sb * SBLK + b : sb * SBLK + b + 1], tpr[0:1, 0, 0, 0:1]
                    )
                    continue
                en8 = None
                for hc in range(NHC):
                    ps = psp.tile([128, SBLK], F32)
                    for ecp in range(EPB):
                        rhs = (
                            tpr[:, ecp, :, :]
                            .bitcast(FP8)
                            .rearrange("p sub (s i) -> p i (sub s)", i=2)
                        )
                        nc.tensor.matmul(
                            ps[:],
                            wdr[:, ecp, :, hc * 128 : (hc + 1) * 128],
                            rhs,
                            start=(ecp == 0),
                            stop=(ecp == EPB - 1),
                            perf_mode=DR,
                        )
                    if stage < 4:
                        nc.vector.tensor_copy(
                            L[0:1, sb * SBLK + b * NHC + hc : sb * SBLK + b * NHC + hc + 1],
                            ps[0:1, 0:1],
                        )
                        continue
                    if fp8dot:
                        jv, ih = divmod(hc, 2)
                        nc.scalar.activation(
                            en8all[:, b, jv, ih],
                            ps[:],
                            AF.Tanh,
                            bias=decb[:, hc, b : b + 1],
                            scale=1.0 / WSCALE,
                        )
                    else:
                        en = enp.tile([128, SBLK], BF16)
                        nc.scalar.activation(
                            en[:],
                            ps[:],
                            AF.Tanh,
                            bias=decb[:, hc, b : b + 1],
                            scale=1.0 / WSCALE,
                        )
                        pending.append((b, hc, en))
                        if len(pending) > 2:
                            pb_, phc, pen = pending.pop(0)
                            nc.tensor.matmul(
                                attps[:],
                                vz[:, pb_, phc, :],
                                pen[:],
                                start=(pb_ == 0 and phc == 0),
                                stop=(pb_ == B_LOC - 1 and phc == NHC - 1),
                            )
            if stage >= 4:
                if fp8dot:
                    prev_v = (en8all, attps, sb)
                else:
                    for pb_, phc, pen in pending:
                        nc.tensor.matmul(
                            attps[:],
                            vz[:, pb_, phc, :],
                            pen[:],
                            start=(pb_ == 0 and phc == 0),
                            stop=(pb_ == B_LOC - 1 and phc == NHC - 1),
                        )
                    prev = (attps, sb)
         if prev_v is not None:
            burst(prev_v)
            prev = (prev_v[1], prev_v[2])
            prev_v = None
         if prev is not None:
            drain(prev)
            prev = None

        # ---- finish softmax: total sums -> reciprocal -> scale -> store ----
        if stage >= 4:
            Ssum = singles.tile([B_LOC, 1], F32)
            nc.vector.tensor_reduce(
                Ssum[:], Ssum4[:], axis=mybir.AxisListType.X, op=mybir.AluOpType.add
            )
            R = singles.tile([B_LOC, 1], F32)
            nc.vector.reciprocal(R[:], Ssum[:])
            O = singles.tile([B_LOC, S], F32)
            nc.vector.tensor_scalar_mul(O[:], Ex[:], R[:])
            nc.sync.dma_start(out=out[:, :], in_=O[:])
        else:
            nc.sync.dma_start(out=out[:, :], in_=L[:])

    nc.compile()
    return nc


def _get_state():
    if _CACHE:
        return _CACHE
    import jax
    from jax.experimental.shard_map import shard_map
    from jax.sharding import Mesh, PartitionSpec
    from concourse import bass2jax, mybir

    nc = _build_kernel()
    bass2jax.install_neuronx_cc_hook()

    partition_name = nc.partition_id_tensor.name if nc.partition_id_tensor else None
    in_names: list[str] = []
    out_names: list[str] = []
    out_avals = []
    zero_shapes = []
    for alloc in nc.m.functions[0].allocations:
        if not isinstance(alloc, mybir.MemoryLocationSet):
            continue
        name = alloc.memorylocations[0].name
        if alloc.kind == "ExternalInput":
            if name != partition_name:
                in_names.append(name)
        elif alloc.kind == "ExternalOutput":
            shape = tuple(alloc.tensor_shape)
            dtype = mybir.dt.np(alloc.dtype)
            out_names.append(name)
            out_avals.append(jax.core.ShapedArray(shape, dtype))
            zero_shapes.append((shape, dtype))
    n_params = len(in_names)
    all_names = list(in_names + out_names)
    if partition_name is not None:
        all_names.append(partition_name)
    all_names = tuple(all_names)

    def _body(*args):
        operands = list(args)
        if partition_name is not None:
            operands.append(bass2jax.partition_id_tensor())
        outs = bass2jax._bass_exec_p.bind(
            *operands,
            out_avals=tuple(out_avals),
            in_names=all_names,
            out_names=tuple(out_names),
            lowering_input_output_aliases=(),
            sim_require_finite=True,
            sim_require_nnan=True,
            nc=nc,
        )
        return tuple(outs)

    devices = jax.devices()[:NCORES]
    mesh = Mesh(np.asarray(devices), ("core",))
    n_outs = len(out_names)
    in_specs = (PartitionSpec("core"),) * (n_params + n_outs)
    out_specs = (PartitionSpec("core"),) * n_outs
    donate = tuple(range(n_params, n_params + n_outs))
    fn = jax.jit(
        shard_map(_body, mesh=mesh, in_specs=in_specs, out_specs=out_specs, check_rep=False),
        donate_argnums=donate,
        keep_unused=True,
    )
    _CACHE.update(
        dict(fn=fn, nc=nc, in_names=in_names, out_names=out_names, zero_shapes=zero_shapes, mesh=mesh)
    )
    return _CACHE


def _concat_inputs(inputs):
    """Build the global (concat over cores on axis 0) arrays in in_names order."""
    st = _get_state()
    per_name = {}
    # per-core shards
    dec_s = inputs["decoder_hide"].reshape(NCORES, B_LOC, H)
    enc_s = inputs["encoder_out"].reshape(NCORES, B_LOC, S, E)
    msk_s = inputs["mask"].reshape(NCORES, B_LOC, S)
    per_name["decoder_hide"] = dec_s.reshape(NCORES * B_LOC, H)
    per_name["encoder_out"] = enc_s.reshape(NCORES * B_LOC, S, E)
    per_name["mask"] = msk_s.reshape(NCORES * B_LOC, S)
    # replicated weights: tile along axis 0
    per_name["W_attn"] = np.tile(inputs["W_attn"], (NCORES, 1))
    per_name["b_attn"] = np.tile(inputs["b_attn"], NCORES)
    per_name["v_w"] = np.tile(inputs["v_w"], NCORES)
    return [np.ascontiguousarray(per_name[n]) for n in st["in_names"]]


def _zero_outs():
    st = _get_state()
    return [
        np.zeros((NCORES * shape[0], *shape[1:]), dtype) for shape, dtype in st["zero_shapes"]
    ]


def kernel(**inputs) -> np.ndarray:
    st = _get_state()
    concat_in = _concat_inputs(inputs)
    outs = st["fn"](*concat_in, *_zero_outs())
    out = np.asarray(outs[st["out_names"].index("out")])
    return out.reshape(B, S)
